# revision 49
# baseline (speedup 1.0000x reference)
"""Multi-head self-attention Bass kernel for TRN2, batch-parallel over 8 cores.

Per-core problem (batch element b): x [T=1024, D=1024], 16 heads, d_k=64.

Precision/cost scheme (cost model charges out_free_rows x cycles(moving
dtype); fp8e4+DoubleRow = 0.5 cyc/row and contracts 2 k-tiles/instr):
  - Projections (K=1024 contractions: QKV and output): operands split
    hi/lo into fp8e4 (a ~= a_hi + a_lo, each e4m3); 3-term product
    a_hi*b_hi + a_lo*b_hi + a_hi*b_lo via DoubleRow pairs -> 12 DR
    instructions per [128,512] psum tile vs 8 bf16 ones: 25% fewer PE
    cycles at ~1e-3 accuracy (better than bf16).
  - S = K^T Q (K=64 contraction) and AV (error-sensitive): fp16.
  - Weights pre-scaled x32 on host (avoids e4m3 subnormals); exp absorbs
    the 32*32 logit scale via ACT scale=1/8192 and folds a /64 range
    shift via bias=-ln(64) so fp16 ET never overflows.

Dataflow (trailing T = transposed layout [feature, token]):
  xh/xl   [D, T]    host-split fp8 hi/lo of x^T
  vg      [T,16*65] V natural (x32) + per-head ones column
  qk      [2D, T]   Q^T,K^T fp16 (x32): DR c-tiles, lhsT=W slices, rhs=x
  ST_h    [Tk, Tq]  = K_h Q_h^T per (head, tk): [128,1024] psum (x1024)
  ET_h    fp16      = exp(ST/8192 - ln64) = e_true/64, one ACT op/tile
  AV      natural:  lhsT=ET[:, tk, q-slice], rhs=vg 65-col slice
                    -> psum [128q, 4*65]; col 64 = sums/64
  O       normalized on DVE (per-partition 64/sums -> onat = 32*o_norm),
          transposed via PE identity-matmul, drained as fp8 hi/lo pair
  y       [T, D]    3-term DR against hi/lo W_o (x32), drained as
                    y = psum * 2^-10 + (b_v @ W_o + b_o)

Schedule: single in-order PE stream, software-pipelined per head:
S(h, tk) tiles feed the ACT exp stream; V tiles (heads 0-1 window) and
QK c-tiles (one per head) are interleaved as PE filler; AV(h-2) runs two
heads behind S(h); output projection at the end.
"""
import math
import numpy as np
import concourse.bacc as bacc
import concourse.mybir as mybir
from concourse.tile import TileContext
from concourse.bass import ts

F32 = mybir.dt.float32
F16 = mybir.dt.float16
F8 = mybir.dt.float8e4
AF = mybir.ActivationFunctionType
DR = mybir.MatmulPerfMode.DoubleRow
MULT = mybir.AluOpType.mult
SUB = mybir.AluOpType.subtract

T = 1024       # tokens per core (one batch element)
D = 1024       # d_model
H = 16         # heads
DK = 64        # head dim
NT = T // 128  # 8 token tiles
ND = D // 128  # 8 d tiles
NP = ND // 2   # 4 k-tile pairs for DoubleRow
NC_T = T // 512  # 2 free-dim chunks of tokens
VW = H * (DK + 1)  # 1040, augmented V width
EXP_SCALE = 1.0 / 8192.0      # undo 32*32 on q,k then * 1/sqrt(dk)
EXP_BIAS = -math.log(64.0)    # et = e_true / 64
Y_SCALE = 2.0 ** -10          # undo 32(o) * 32(Wo)


def build_nc(repeat=1):
    nc = bacc.Bacc(None, target_bir_lowering=False, debug=False)

    xhd = nc.dram_tensor("xh", [D, T], F8, kind="ExternalInput")
    xld = nc.dram_tensor("xl", [D, T], F8, kind="ExternalInput")
    wqkh = nc.dram_tensor("wqkh", [D, 2 * D], F8, kind="ExternalInput")
    wqkl = nc.dram_tensor("wqkl", [D, 2 * D], F8, kind="ExternalInput")
    bqkc = nc.dram_tensor("bqkc", [128, 2 * ND], F32, kind="ExternalInput")
    wvhd = nc.dram_tensor("wvh", [D, VW], F8, kind="ExternalInput")
    wvld = nc.dram_tensor("wvl", [D, VW], F8, kind="ExternalInput")
    bvd = nc.dram_tensor("bv", [1, VW], F16, kind="ExternalInput")
    wohd = nc.dram_tensor("woh", [D, D], F8, kind="ExternalInput")
    wold = nc.dram_tensor("wol", [D, D], F8, kind="ExternalInput")
    bord = nc.dram_tensor("bor", [128, D], F32, kind="ExternalInput")
    onesd = nc.dram_tensor("onesd", [1, 128], F16, kind="ExternalInput")
    identd = nc.dram_tensor("identd", [128, 128], F16, kind="ExternalInput")
    ebiasd = nc.dram_tensor("ebias", [128, 1], F32, kind="ExternalInput")
    y = nc.dram_tensor("y", [T, D], F32, kind="ExternalOutput")

    xh_r = xhd.rearrange("(dt p) t -> p dt t", p=128)
    xl_r = xld.rearrange("(dt p) t -> p dt t", p=128)
    wqkh_r = wqkh.rearrange("(dt p) c -> p dt c", p=128)
    wqkl_r = wqkl.rearrange("(dt p) c -> p dt c", p=128)
    wvh_r = wvhd.rearrange("(dt p) c -> p dt c", p=128)
    wvl_r = wvld.rearrange("(dt p) c -> p dt c", p=128)
    woh_r = wohd.rearrange("(dt p) c -> p dt c", p=128)
    wol_r = wold.rearrange("(dt p) c -> p dt c", p=128)

    with TileContext(nc) as tc:
      for _rep in range(repeat):
        with (
            tc.tile_pool(name="res", bufs=1) as res,
            tc.tile_pool(name="wcp", bufs=4) as wcp,
            tc.tile_pool(name="etp", bufs=3) as etp,
            tc.tile_pool(name="onp", bufs=2) as onp,
            tc.tile_pool(name="invp", bufs=2) as invp,
            tc.tile_pool(name="yp", bufs=5) as yp,
            tc.tile_pool(name="ystp", bufs=1) as ystp,
            tc.tile_pool(name="ottp", bufs=2) as ottp,
            tc.tile_pool(name="psW", bufs=2, space="PSUM") as psW,
            tc.tile_pool(name="psAV", bufs=1, space="PSUM") as psAV,
        ):
            # ---- prelude: constants + input DMAs, spread over 4 queues ----
            xth = res.tile([128, ND, T], F8)
            xtl = res.tile([128, ND, T], F8)
            wvth = res.tile([128, ND, VW], F8)
            wvtl = res.tile([128, ND, VW], F8)
            wcsh, wcsl = {}, {}

            def wc_dma(c, q=None):
                qh = q or nc.sync
                wcsh[c] = wcp.tile([128, ND, 128], F8, tag="wqkh",
                                   name=f"wch_{c}")
                wcsl[c] = wcp.tile([128, ND, 128], F8, tag="wqkl",
                                   name=f"wcl_{c}")
                qh.dma_start(wcsh[c][:], wqkh_r[:, :, ts(c, 128)])
                qh.dma_start(wcsl[c][:], wqkl_r[:, :, ts(c, 128)])

            # prelude DMAs on the two HWDGE queues (SP/ACT), interleaved in
            # first-use order; gpsimd uses slow SWDGE (~1us serial setup
            # per DMA) so it only gets non-critical constants
            wcsh[0] = wcp.tile([128, ND, 128], F8, tag="wqkh", name="wch_0")
            wcsl[0] = wcp.tile([128, ND, 128], F8, tag="wqkl", name="wcl_0")
            wcsh[ND] = wcp.tile([128, ND, 128], F8, tag="wqkh",
                                name=f"wch_{ND}")
            wcsl[ND] = wcp.tile([128, ND, 128], F8, tag="wqkl",
                                name=f"wcl_{ND}")
            # hi weights and hi x land first so the opening A-term (hi*hi)
            # matmuls can start as early as possible; lo tensors follow for
            # the B/C correction terms; big x transfers split across the
            # two HWDGE queues to halve queue-serial latency
            nc.sync.dma_start(wcsh[0][:, 0:2, :], wqkh_r[:, 0:2, ts(0, 128)])
            nc.scalar.dma_start(xth[:, 0:2, 0:512], xh_r[:, 0:2, 0:512])
            nc.sync.dma_start(wcsh[0][:, 2:ND, :], wqkh_r[:, 2:ND, ts(0, 128)])
            nc.scalar.dma_start(xth[:, 2:4, 0:512], xh_r[:, 2:4, 0:512])
            nc.sync.dma_start(wcsh[ND][:], wqkh_r[:, :, ts(ND, 128)])
            nc.scalar.dma_start(xth[:, 4:ND, 0:512], xh_r[:, 4:ND, 0:512])
            nc.sync.dma_start(wcsl[0][:], wqkl_r[:, :, ts(0, 128)])
            nc.sync.dma_start(wcsl[ND][:], wqkl_r[:, :, ts(ND, 128)])
            nc.scalar.dma_start(xth[:, 0:4, 512:T], xh_r[:, 0:4, 512:T])
            nc.scalar.dma_start(xth[:, 4:ND, 512:T], xh_r[:, 4:ND, 512:T])
            nc.sync.dma_start(xtl[:, 0:4, 0:512], xl_r[:, 0:4, 0:512])
            nc.sync.dma_start(xtl[:, 4:ND, 0:512], xl_r[:, 4:ND, 0:512])
            nc.scalar.dma_start(xtl[:, 0:4, 512:T], xl_r[:, 0:4, 512:T])
            nc.sync.dma_start(xtl[:, 4:ND, 512:T], xl_r[:, 4:ND, 512:T])
            bqk_t = res.tile([128, 2 * ND], F32)
            bv_t = res.tile([1, VW], F16)
            for d in range(0, ND, 2):
                q = nc.sync if d < 4 else nc.scalar
                q.dma_start(wvth[:, d:d + 2, :], wvh_r[:, d:d + 2, :])
            for d in range(0, ND, 2):
                q = nc.sync if d < 4 else nc.scalar
                q.dma_start(wvtl[:, d:d + 2, :], wvl_r[:, d:d + 2, :])
            wc_dma(1)
            wc_dma(ND + 1)
            nc.gpsimd.dma_start(bqk_t[:], bqkc[:])
            ones_t = res.tile([1, 128], F16)
            nc.gpsimd.dma_start(ones_t[:], onesd[:])
            nc.gpsimd.dma_start(bv_t[:], bvd[:])
            ident_t = res.tile([128, 128], F16)
            nc.gpsimd.dma_start(ident_t[:], identd[:])
            bor_t = res.tile([128, D], F32)
            nc.gpsimd.dma_start(bor_t[:], bord[:])
            ebias_t = res.tile([128, 1], F32)
            nc.gpsimd.dma_start(ebias_t[:], ebiasd[:])

            qk = res.tile([128, 2 * ND, T], F16)
            vg = res.tile([128, NT, VW], F16)
            oth = res.tile([128, ND, T], F8)
            otl = res.tile([128, ND, T], F8)
            wo_th = res.tile([128, ND, D], F8)
            wo_tl = res.tile([128, ND, D], F8)

            # ---- PE work units (quanta ~0.5-1.5us each) ----
            # 3-term hi/lo fp8 DoubleRow product: out += a.T @ b where
            # a ~= ah+al (stationary), b ~= bh+bl (moving), k-pair p.
            def dr3(pp, ah, al, bh, bl, p, asl, bsl, n0, ntot):
                d = 2 * p
                for i, (at, bt) in enumerate(
                        ((ah, bh), (al, bh), (ah, bl))):
                    nc.tensor.matmul(
                        pp, at[:, d:d + 2, asl], bt[:, d:d + 2, bsl],
                        start=(n0 + i == 0), stop=(n0 + i == ntot - 1),
                        perf_mode=DR)

            def qkc_half(c, tq):
                pp = psW.tile([128, 512], F32, tag="half",
                              name=f"pqk_{c}_{tq}")
                for p in range(NP):
                    dr3(pp[:], wcsh[c], wcsl[c], xth, xtl, p,
                        slice(None), ts(tq, 512), 3 * p, 3 * NP)
                nc.vector.tensor_scalar_add(qk[:, c, ts(tq, 512)], pp[:],
                                            bqk_t[:, c:c + 1])

            vchunks = [[(0, 512)], [(512, 512), (1024, VW - 1024)]]

            def vt_half(t, half):
                # softmax rows sum to 1, so the V bias reduces to a constant
                # +bv per output row, folded into the final y bias instead;
                # only the per-head ones columns (for the softmax sums) need
                # the K=1 matmul here, on a 65-strided view
                for off, w in vchunks[half]:
                    pp = psW.tile([128, 512], F32, tag="half",
                                  name=f"pv_{t}_{off}")
                    for p in range(NP):
                        dr3(pp[:, :w], xth, xtl, wvth, wvtl, p,
                            ts(t, 128), slice(off, off + w), 3 * p, 3 * NP)
                    # the ones columns got exactly 0 from the DR loop (their
                    # W columns are zero), so overwrite them as an own group
                    o0 = (64 - off) % 65
                    nc.tensor.matmul(pp[:, o0:w:65], ones_t[:],
                                     bv_t[:, off + o0:off + w:65],
                                     start=True, stop=True,
                                     skip_group_check=True)
                    nc.vector.tensor_copy(vg[:, t, off:off + w], pp[:, :w])

            def s_tile(h, tk, et_h):
                qi, ki = h // 2, ND + h // 2
                b0 = 64 * (h % 2)
                ps = psW.tile([128, 1024], F32, tag="wide",
                              name=f"ps_{h}_{tk}")
                for tq in range(NC_T):
                    nc.tensor.matmul(
                        ps[:, ts(tq, 512)],
                        qk[b0:b0 + DK, ki, ts(tk, 128)],
                        qk[b0:b0 + DK, qi, ts(tq, 512)],
                        start=True, stop=True, tile_position=(b0, 0))
                nc.scalar.activation(et_h[:, tk, :], ps[:], AF.Exp,
                                     scale=EXP_SCALE, bias=ebias_t[:, 0:1])

            onat = {}   # (pair, qt) -> packed O-natural tile (32*o_norm)
            invs = {}   # h -> per-q-token 64/sum tile

            def av_g(h, g, et_h, act_muls=False):
                hp, sub = h // 2, h % 2
                if g == 0:
                    if sub == 0:
                        for qt in range(NT):
                            onat[(hp, qt)] = onp.tile([128, 128], F16,
                                                      tag=f"on{qt}",
                                                      name=f"onat_{hp}_{qt}")
                    invs[h] = invp.tile([128, NT], F32, tag="inv",
                                        name=f"inv_{h}")
                inv = invs[h]
                pool, tag = (psW, "half") if h == 14 else (psAV, f"av{g}")
                pav = pool.tile([128, 512], F32, tag=tag,
                                name=f"pav_{h}_{g}")
                for ql in range(4):
                    qt = g * 4 + ql
                    for tk in range(NT):
                        nc.tensor.matmul(
                            pav[:, ql * 65:(ql + 1) * 65],
                            et_h[:, tk, ts(qt, 128)],
                            vg[:, tk, h * 65:(h + 1) * 65],
                            start=(tk == 0), stop=(tk == NT - 1))
                nc.vector.reciprocal(inv[:, ts(g, 4)],
                                     pav[:, 64:4 * 65:65])
                for ql in range(4):
                    qt = g * 4 + ql
                    dst = onat[(hp, qt)][:, sub * DK:(sub + 1) * DK]
                    src = pav[:, ql * 65:ql * 65 + DK]
                    if act_muls:
                        # pipeline tail: ACT is idle once the last exp is
                        # done, so normalize there and keep DVE free for
                        # the tp drains (Copy shares the Exp act table)
                        nc.scalar.mul(dst, src, inv[:, qt:qt + 1])
                    else:
                        nc.vector.tensor_scalar_mul(dst, src,
                                                    inv[:, qt:qt + 1])

            ysts = {}  # (t, oc) -> staged fp16 partial y (k-pairs 0..1)

            def oproj_pairs(py, t, oc, w, p0, p1, n0, ntot):
                n = n0
                for p in range(p0, p1):
                    dr3(py, oth, otl, wo_th, wo_tl, p,
                        ts(t, 128), slice(oc * 512, oc * 512 + w), n, ntot)
                    n += 3

            deep_yst = set()

            def oproj_partial(t, oc, np1=2):
                ph = psW.tile([128, 512], F32, tag="half",
                              name=f"pyp_{t}_{oc}")
                oproj_pairs(ph[:], t, oc, 512, 0, np1, 0, 3 * np1)
                yst = ystp.tile([128, 512], F16, tag=f"yst{t}_{oc}",
                                name=f"yst_{t}_{oc}")
                nc.vector.tensor_copy(yst[:], ph[:])
                ysts[(t, oc)] = yst
                if np1 == 3:
                    deep_yst.add((t, oc))

            def tp_q(p, g):
                # transpose O-natural pair tiles back to feature-major, then
                # drain the 512-wide result as an fp8 hi/lo pair for the DR
                # output projection.  Early pairs ride the idle DMA engines'
                # xbar transpose (16-bit, 112ns/tile); the last two pairs
                # stay on PE identity-matmuls (shorter latency chain at the
                # pipeline tail): 4 matmuls into ONE psum bank as disjoint
                # 128-col groups, drained once.
                osl = ts(g, 512)
                if p < 6:
                    ott = ottp.tile([128, 512], F16, tag="ott",
                                    name=f"ott_{p}_{g}")
                    for i in range(4):
                        qt = g * 4 + i
                        nc.sync.dma_start_transpose(ott[:, ts(i, 128)],
                                                    onat[(p, qt)][:])
                    src = ott[:]
                else:
                    hold = psAV.tile([128, 512], F32, tag=f"av{g}",
                                     name=f"ptp_{p}_{g}")
                    for i in range(4):
                        qt = g * 4 + i
                        nc.tensor.matmul(hold[:, ts(i, 128)],
                                         onat[(p, qt)][:],
                                         ident_t[:], start=True, stop=True,
                                         skip_group_check=(i > 0))
                    src = hold[:]
                nc.vector.tensor_copy(oth[:, p, osl], src)
                nc.vector.scalar_tensor_tensor(
                    otl[:, p, osl], src, 1.0, oth[:, p, osl], MULT, SUB)

            # ---- main software-pipelined stream ----
            # per-head slot plans: slots[tk] = list of filler callables
            # emitted right after S(h, tk); emission order == PE order.
            # AV lags two heads behind S (exp of head h-2 is complete);
            # V must fully precede the first AV read of vg.
            def qkc_open():
                # the four opening Q/K psum tiles (c0/c8 x both chunks)
                # accumulate together, emitted in DMA-arrival order: all
                # A-terms (hi*hi) first, then B (w_lo), then C (x_lo), so
                # every arriving transfer immediately feeds matmuls
                pa = psW.tile([128, 512], F32, tag="half", name="pqk_0_0")
                pb = psW.tile([128, 512], F32, tag="half", name=f"pqk_{ND}_0")
                pw = psW.tile([128, 1024], F32, tag="wide", name="pqk_t1")
                t0 = [(pa, 0, 0), (pb, ND, 0)]
                t1 = [(pw[:, 0:512], 0, 1), (pw[:, 512:1024], ND, 1)]
                for group, xt_, w_, st in (
                        (t0, xth, wcsh, True), (t0, xth, wcsl, False),
                        (t1, xth, wcsh, True), (t1, xth, wcsl, False),
                        (t0, xtl, wcsh, False), (t1, xtl, wcsh, False)):
                    for pp, c, tq in group:
                        for p in range(NP):
                            nc.tensor.matmul(
                                pp, w_[c][:, 2 * p:2 * p + 2, :],
                                xt_[:, 2 * p:2 * p + 2, ts(tq, 512)],
                                start=(st and p == 0),
                                stop=(xt_ is xtl and p == NP - 1),
                                perf_mode=DR)
                        if xt_ is xtl:
                            nc.vector.tensor_scalar_add(
                                qk[:, c, ts(tq, 512)], pp, bqk_t[:, c:c + 1])

            qkc_open()
            et_tiles = {}

            def avq(h, g):
                return lambda: av_g(h, g, et_tiles[h])

            def qkq(c, tq):
                return lambda: qkc_half(c, tq)

            def vtq(t, half):
                return lambda: vt_half(t, half)

            def tpq(p, g):
                return lambda: tp_q(p, g)

            def wcq(c):
                return lambda: wc_dma(c)

            def woq(d):
                def f():
                    nc.sync.dma_start(wo_th[:, d:d + 2, :],
                                      woh_r[:, d:d + 2, :])
                    nc.sync.dma_start(wo_tl[:, d:d + 2, :],
                                      wol_r[:, d:d + 2, :])
                return f

            # output-tile order: partials staged h10-h15, finishers at the
            # tail (pair-2 + ident first, pair-3 after tp(7) lands)
            _order = [(t, oc) for t in range(NT) for oc in range(NC_T)]

            def ppq(i, np1=2):
                t, oc = _order[i]
                return lambda: oproj_partial(t, oc, np1)

            opened = {}

            def fin_open(i, py):
                t, oc = _order[i]
                if (t, oc) in deep_yst:
                    nc.tensor.matmul(py, ident_t[:], ysts[(t, oc)][:],
                                     start=True, stop=False)
                else:
                    oproj_pairs(py, t, oc, 512, 2, 3, 0, 99)
                    nc.tensor.matmul(py, ident_t[:], ysts[(t, oc)][:],
                                     start=False, stop=False)
                opened[i] = py

            def foq(i):
                # finisher pair-2 block opened inside h15 on a freed half
                # bank: fills the S(15) exp-latency slots and shortens the
                # post-loop tail
                def f():
                    ph = psW.tile([128, 512], F32, tag="half",
                                  name=f"pfo_{i}")
                    fin_open(i, ph[:])
                return f

            for h in range(H):
                et_tiles[h] = etp.tile([128, NT, T], F16, tag="et",
                                       name=f"et_{h}")
                slots = [[] for _ in range(NT)]
                if h == 0:
                    slots[0] = [qkq(1, 0)]
                    slots[1] = [qkq(1, 1)]
                    slots[2] = [vtq(0, 0)]
                    slots[3] = [vtq(0, 1)]
                    slots[4] = [vtq(1, 0)]
                    slots[5] = [vtq(1, 1)]
                    slots[6] = [vtq(2, 0)]
                    slots[7] = [vtq(2, 1)]
                elif h == 1:
                    slots[0] = [vtq(3, 0)]
                    slots[1] = [vtq(3, 1)]
                    slots[2] = [wcq(2), qkq(ND + 1, 0)]
                    slots[3] = [qkq(ND + 1, 1)]
                    slots[4] = [vtq(4, 0)]
                    slots[5] = [vtq(4, 1)]
                    slots[6] = [vtq(5, 0)]
                    slots[7] = [vtq(5, 1)]
                elif h == 2:
                    slots[0] = [vtq(6, 0)]
                    slots[1] = [vtq(6, 1)]
                    slots[2] = [wcq(ND + 2), vtq(7, 0)]
                    slots[3] = [vtq(7, 1)]
                    slots[4] = [qkq(2, 0)]
                    slots[5] = [qkq(2, 1)]
                    slots[6] = [avq(0, 0)]
                    slots[7] = [avq(0, 1)]
                elif h == 3:
                    slots[0] = [avq(1, 0)]
                    slots[1] = [qkq(ND + 2, 0)]
                    slots[2] = [avq(1, 1)]
                    slots[3] = [qkq(ND + 2, 1)]
                    slots[4] = [tpq(0, 0)]
                    slots[5] = [tpq(0, 1)]
                    slots[6] = [wcq(3)]
                    slots[7] = [wcq(ND + 3)]
                elif h == 4:
                    slots[0] = [avq(2, 0)]
                    slots[1] = [qkq(3, 0)]
                    slots[2] = [avq(2, 1)]
                    slots[3] = [qkq(3, 1)]
                    slots[4] = [qkq(ND + 3, 0)]
                    slots[5] = [wcq(4)]
                    slots[6] = [wcq(ND + 4)]
                elif h == 5:
                    slots[0] = [avq(3, 0)]
                    slots[1] = [qkq(ND + 3, 1)]
                    slots[2] = [avq(3, 1)]
                    slots[3] = [qkq(4, 0)]
                    slots[4] = [tpq(1, 0)]
                    slots[5] = [qkq(4, 1)]
                    slots[6] = [tpq(1, 1)]
                    slots[7] = [wcq(5)]
                elif h == 6:
                    slots[0] = [avq(4, 0)]
                    slots[1] = [qkq(ND + 4, 0)]
                    slots[2] = [avq(4, 1)]
                    slots[3] = [qkq(ND + 4, 1)]
                    slots[4] = [qkq(5, 0)]
                    slots[5] = [wcq(ND + 5)]
                    slots[6] = [woq(0)]
                    slots[7] = []
                elif h == 7:
                    slots[0] = [avq(5, 0)]
                    slots[1] = [qkq(5, 1)]
                    slots[2] = [avq(5, 1)]
                    slots[3] = [qkq(ND + 5, 0)]
                    slots[4] = [tpq(2, 0)]
                    slots[5] = [qkq(ND + 5, 1)]
                    slots[6] = [tpq(2, 1)]
                    slots[7] = [wcq(6), wcq(ND + 6)]
                elif h == 8:
                    slots[0] = [avq(6, 0)]
                    slots[1] = [qkq(6, 0)]
                    slots[2] = [avq(6, 1)]
                    slots[3] = [qkq(6, 1)]
                    slots[4] = [qkq(ND + 6, 0)]
                    slots[5] = [wcq(7)]
                    slots[6] = [woq(2)]
                elif h == 9:
                    slots[0] = [avq(7, 0)]
                    slots[1] = [qkq(ND + 6, 1)]
                    slots[2] = [avq(7, 1)]
                    slots[3] = [qkq(7, 0)]
                    slots[4] = [tpq(3, 0)]
                    slots[5] = [qkq(7, 1)]
                    slots[6] = [tpq(3, 1)]
                    slots[7] = [wcq(ND + 7)]
                elif h == 10:
                    slots[0] = [avq(8, 0)]
                    slots[1] = [qkq(ND + 7, 0)]
                    slots[2] = [avq(8, 1)]
                    slots[3] = [qkq(ND + 7, 1)]
                    slots[4] = [ppq(0)]
                    slots[5] = [ppq(1)]
                    slots[7] = [woq(4)]
                elif h == 11:
                    slots[0] = [avq(9, 0)]
                    slots[1] = [ppq(2)]
                    slots[2] = [avq(9, 1)]
                    slots[3] = [ppq(3)]
                    slots[4] = [tpq(4, 0)]
                    slots[5] = [ppq(4)]
                    slots[6] = [tpq(4, 1)]
                    slots[7] = [woq(6)]
                elif h == 12:
                    slots[0] = [avq(10, 0)]
                    slots[1] = [ppq(5)]
                    slots[2] = [avq(10, 1)]
                    slots[3] = [ppq(6)]
                    slots[4] = [ppq(7)]
                    slots[5] = [ppq(8)]
                elif h == 13:
                    slots[0] = [avq(11, 0)]
                    slots[1] = [ppq(9)]
                    slots[2] = [avq(11, 1)]
                    slots[3] = [ppq(10)]
                    slots[4] = [tpq(5, 0)]
                    slots[5] = [ppq(11)]
                    slots[6] = [tpq(5, 1)]
                elif h == 14:
                    slots[0] = [avq(12, 0)]
                    slots[1] = [ppq(12, 3)]
                    slots[2] = [avq(12, 1)]
                    slots[3] = [ppq(13, 3)]
                    slots[5] = [ppq(14, 3)]
                    slots[7] = [ppq(15, 3)]
                else:  # h == 15
                    slots[0] = [avq(13, 0)]
                    slots[1] = [avq(14, 0)]
                    slots[2] = [avq(13, 1)]
                    slots[3] = [avq(14, 1)]
                    slots[4] = [tpq(6, 0)]
                    slots[6] = [tpq(6, 1)]
                for tk in range(NT):
                    s_tile(h, tk, et_tiles[h])
                    for fn in slots[tk]:
                        fn()

            # ---- pipeline tail + output projection finishers ----
            # av(15) normalizes on the now-idle ACT engine; the first four
            # finishers' pair-2 blocks and ident re-injects run while the
            # av(15) -> tp(7) chain completes, pair-3 closes after
            _dq = [0]

            def fin_drain(t, oc, py):
                yt = yp.tile([128, 512], F32, tag="yt",
                             name=f"yt_{t}_{oc}")
                # scale + bias during the drain (bias includes the folded
                # V-bias term b_v @ W_o); the first half of the finisher
                # stream drains on idle GPSIMD so DVE stays fresh for the
                # span-critical last drains
                nc.vector.scalar_tensor_tensor(
                    yt[:], py, Y_SCALE, bor_t[:, ts(oc, 512)],
                    MULT, mybir.AluOpType.add)
                q = nc.sync if _dq[0] % 2 == 0 else nc.scalar
                _dq[0] += 1
                q.dma_start(y[ts(t, 128), ts(oc, 512)], yt[:])

            def fin_close(i):
                t, oc = _order[i]
                py = opened.pop(i)
                oproj_pairs(py, t, oc, 512, 3, NP, 1, 4)
                fin_drain(t, oc, py)

            def fin_full(i, py):
                t, oc = _order[i]
                if (t, oc) in deep_yst:
                    nc.tensor.matmul(py, ident_t[:], ysts[(t, oc)][:],
                                     start=True, stop=False)
                else:
                    oproj_pairs(py, t, oc, 512, 2, 3, 0, 99)
                    nc.tensor.matmul(py, ident_t[:], ysts[(t, oc)][:],
                                     start=False, stop=False)
                oproj_pairs(py, t, oc, 512, 3, NP, 1, 4)
                fin_drain(t, oc, py)

            def wide_halves(name):
                pw = psW.tile([128, 1024], F32, tag="wide", name=name)
                return pw[:, 0:512], pw[:, 512:1024]

            av_g(H - 1, 0, et_tiles[H - 1], act_muls=True)
            wa0, wa1 = wide_halves("pfin_a")
            fin_open(0, wa0)
            fin_open(1, wa1)
            av_g(H - 1, 1, et_tiles[H - 1], act_muls=True)
            wb0, wb1 = wide_halves("pfin_b")
            fin_open(2, wb0)
            tp_q(H // 2 - 1, 0)
            fin_open(3, wb1)
            tp_q(H // 2 - 1, 1)
            ph0 = psW.tile([128, 512], F32, tag="half", name="pfo_0")
            fin_open(4, ph0[:])
            ph1 = psW.tile([128, 512], F32, tag="half", name="pfo_1")
            fin_open(5, ph1[:])
            for i in range(6):
                fin_close(i)

            def last_tile_split(t, oc):
                # final tile: column halves on INDEPENDENT psum tags
                # (dead wide bank + half bank) so each half's drain+DMA
                # chain pipelines with the other half's matmuls
                yt = yp.tile([128, 512], F32, tag="yt", name="yt_last")
                for hh in range(2):
                    sl = slice(hh * 256, (hh + 1) * 256)
                    co = oc * 512 + hh * 256
                    if hh == 0:
                        ph = psW.tile([128, 1024], F32, tag="wide",
                                      name="pyl_0")[:, 0:256]
                    else:
                        ph = psW.tile([128, 512], F32, tag="half",
                                      name="pyl_1")[:, 0:256]
                    p0 = 3 if (t, oc) in deep_yst else 2
                    n = 0
                    for p in range(p0, NP):
                        d = 2 * p
                        for (ot_, wo_) in ((oth, wo_th), (otl, wo_th),
                                           (oth, wo_tl)):
                            nc.tensor.matmul(
                                ph, ot_[:, d:d + 2, ts(t, 128)],
                                wo_[:, d:d + 2, co:co + 256],
                                start=(n == 0), stop=False, perf_mode=DR)
                            n += 1
                    nc.tensor.matmul(ph, ident_t[:], ysts[(t, oc)][:, sl],
                                     start=False, stop=True)
                    nc.vector.scalar_tensor_tensor(
                        yt[:, sl], ph, Y_SCALE, bor_t[:, co:co + 256],
                        MULT, mybir.AluOpType.add)
                    q = nc.scalar if hh == 0 else nc.sync
                    q.dma_start(y[ts(t, 128), co:co + 256], yt[:, sl])

            for i_ in range(6, len(_order)):
                    if i_ == len(_order) - 1:
                        last_tile_split(*_order[i_])
                        continue
                    # rotate over the dead S-tile banks (wide), the half
                    # banks and the freed AV banks so yt drains never gate
                    # the matmuls
                    k = i_ % 3
                    if k == 0:
                        py = psW.tile([128, 1024], F32, tag="wide",
                                      name=f"pfin_{i_}")[:, 0:512]
                    elif k == 1:
                        py = psW.tile([128, 512], F32, tag="half",
                                      name=f"pfin_{i_}")
                    else:
                        py = psAV.tile([128, 512], F32,
                                       tag=f"av{(i_ // 3) % 2}",
                                       name=f"pfin_{i_}")
                    fin_full(i_, py[:])

    nc.finalize()
    return nc


def prep_in_maps(x, W_qkv, b_qkv, W_o, b_o):
    """Host-side sharding: batch-parallel, one batch element per core.
    Splits x and all weights into fp8e4 hi/lo pairs (weights pre-scaled
    x32 to dodge e4m3 subnormals); computes the folded output bias."""
    F8N = mybir.dt.np(F8)

    def hilo8(a):
        a = np.ascontiguousarray(a, dtype=np.float32)
        hi = a.astype(F8N)
        lo = (a - hi.astype(np.float32)).astype(F8N)
        return hi, lo

    B = x.shape[0]
    W_qk = np.asarray(W_qkv[:, :2 * D], np.float32) * 32.0
    wqk_h, wqk_l = hilo8(W_qk)
    b_qkc = np.ascontiguousarray(
        (np.asarray(b_qkv[:2 * D], np.float32) * 32.0).reshape(2 * ND, 128).T)
    W_vo = np.asarray(W_qkv[:, 2 * D:], np.float32)   # [D, D] V weights
    b_vo = np.asarray(b_qkv[2 * D:], np.float32)
    wv_aug = np.zeros((D, VW), np.float32)
    bv_aug = np.zeros((1, VW), np.float16)
    for h in range(H):
        wv_aug[:, h * (DK + 1):h * (DK + 1) + DK] = \
            W_vo[:, h * DK:(h + 1) * DK] * 32.0
        bv_aug[0, h * (DK + 1) + DK] = 1.0
    wv_h, wv_l = hilo8(wv_aug)
    wo_h, wo_l = hilo8(np.asarray(W_o, np.float32) * 32.0)
    ones = np.ones((1, 128), np.float16)
    ident = np.eye(128, dtype=np.float16)
    ebias = np.full((128, 1), EXP_BIAS, np.float32)
    # folded output bias: y = (o_norm + b_v) @ W_o + b_o
    b_eff = (b_vo @ np.asarray(W_o, np.float32)
             + np.asarray(b_o, np.float32)).astype(np.float32)
    b_or = np.ascontiguousarray(
        np.broadcast_to(b_eff.reshape(1, -1), (128, D)))
    in_maps = []
    for b in range(B):
        xT = np.ascontiguousarray(np.asarray(x[b], np.float32).T)
        xh, xl = hilo8(xT)
        in_maps.append({
            "xh": xh, "xl": xl,
            "wqkh": wqk_h, "wqkl": wqk_l, "bqkc": b_qkc,
            "wvh": wv_h, "wvl": wv_l, "bv": bv_aug,
            "woh": wo_h, "wol": wo_l, "bor": b_or,
            "onesd": ones, "identd": ident, "ebias": ebias,
        })
    return in_maps


# ---------------------------------------------------------------------------
# Self-contained SPMD runner (axon PJRT path) and the graded entry point.
# ---------------------------------------------------------------------------
import jax as _jax


_CACHE = {}


def _make_runner(nc, n_cores=8):
    from jax.sharding import Mesh, PartitionSpec
    from jax.experimental.shard_map import shard_map
    from concourse import bass2jax

    bass2jax.install_neuronx_cc_hook()
    partition_name = nc.partition_id_tensor.name if nc.partition_id_tensor else None
    in_names, out_names, out_avals, zero_outs = [], [], [], []
    for alloc in nc.m.functions[0].allocations:
        if not isinstance(alloc, mybir.MemoryLocationSet):
            continue
        name = alloc.memorylocations[0].name
        if alloc.kind == "ExternalInput":
            if name != partition_name:
                in_names.append(name)
        elif alloc.kind == "ExternalOutput":
            shape = tuple(alloc.tensor_shape)
            dtype = mybir.dt.np(alloc.dtype)
            out_names.append(name)
            out_avals.append(_jax.core.ShapedArray(shape, dtype))
            zero_outs.append(np.zeros(shape, dtype))
    n_params = len(in_names)
    all_in_names = list(in_names) + list(out_names)
    if partition_name is not None:
        all_in_names.append(partition_name)

    def _body(*args):
        operands = list(args)
        if partition_name is not None:
            operands.append(bass2jax.partition_id_tensor())
        return tuple(bass2jax._bass_exec_p.bind(
            *operands,
            out_avals=tuple(out_avals),
            in_names=tuple(all_in_names),
            out_names=tuple(out_names),
            lowering_input_output_aliases=(),
            sim_require_finite=True,
            sim_require_nnan=True,
            nc=nc,
        ))

    devices = _jax.devices()[:n_cores]
    mesh = Mesh(np.asarray(devices), ("core",))
    nin = n_params + len(out_names)
    sharded = _jax.jit(
        shard_map(_body, mesh=mesh,
                  in_specs=(PartitionSpec("core"),) * nin,
                  out_specs=(PartitionSpec("core"),) * len(out_names),
                  check_rep=False),
        keep_unused=True,
    )

    def run(in_maps):
        concat_in = [
            np.concatenate([np.asarray(m[name]) for m in in_maps], axis=0)
            for name in in_names
        ]
        concat_zeros = [
            np.zeros((n_cores * z.shape[0], *z.shape[1:]), z.dtype)
            for z in zero_outs
        ]
        out_arrs = [np.asarray(o) for o in sharded(*concat_in, *concat_zeros)]
        return [
            {name: out_arrs[i].reshape(n_cores, *out_avals[i].shape)[c]
             for i, name in enumerate(out_names)}
            for c in range(n_cores)
        ]

    return run


def kernel(x, W_qkv, b_qkv, W_o, b_o):
    """Full-input entry point: shards batch across the 8 NeuronCores,
    runs the Bass MHA kernel SPMD, gathers the full output."""
    x = np.ascontiguousarray(np.asarray(x, np.float32))
    W_qkv = np.asarray(W_qkv, np.float32)
    b_qkv = np.asarray(b_qkv, np.float32)
    W_o = np.asarray(W_o, np.float32)
    b_o = np.asarray(b_o, np.float32)
    B = x.shape[0]
    assert x.shape == (8, T, D), f"unexpected x shape {x.shape}"

    if "run" not in _CACHE:
        nc = build_nc()
        _CACHE["run"] = _make_runner(nc, n_cores=8)
    run = _CACHE["run"]

    in_maps = prep_in_maps(x, W_qkv, b_qkv, W_o, b_o)
    res = run(in_maps)
    out = np.stack([res[b]["y"] for b in range(B)]).astype(np.float32)
    return out


# revision 52
# speedup vs baseline: 1.0023x; 1.0023x over previous
"""Multi-head self-attention Bass kernel for TRN2, batch-parallel over 8 cores.

Per-core problem (batch element b): x [T=1024, D=1024], 16 heads, d_k=64.

Precision/cost scheme (cost model charges out_free_rows x cycles(moving
dtype); fp8e4+DoubleRow = 0.5 cyc/row and contracts 2 k-tiles/instr):
  - Projections (K=1024 contractions: QKV and output): operands split
    hi/lo into fp8e4 (a ~= a_hi + a_lo, each e4m3); 3-term product
    a_hi*b_hi + a_lo*b_hi + a_hi*b_lo via DoubleRow pairs -> 12 DR
    instructions per [128,512] psum tile vs 8 bf16 ones: 25% fewer PE
    cycles at ~1e-3 accuracy (better than bf16).
  - S = K^T Q (K=64 contraction) and AV (error-sensitive): fp16.
  - Weights pre-scaled x32 on host (avoids e4m3 subnormals); exp absorbs
    the 32*32 logit scale via ACT scale=1/8192 and folds a /64 range
    shift via bias=-ln(64) so fp16 ET never overflows.

Dataflow (trailing T = transposed layout [feature, token]):
  xh/xl   [D, T]    host-split fp8 hi/lo of x^T
  vg      [T,16*65] V natural (x32) + per-head ones column
  qk      [2D, T]   Q^T,K^T fp16 (x32): DR c-tiles, lhsT=W slices, rhs=x
  ST_h    [Tk, Tq]  = K_h Q_h^T per (head, tk): [128,1024] psum (x1024)
  ET_h    fp16      = exp(ST/8192 - ln64) = e_true/64, one ACT op/tile
  AV      natural:  lhsT=ET[:, tk, q-slice], rhs=vg 65-col slice
                    -> psum [128q, 4*65]; col 64 = sums/64
  O       normalized on DVE (per-partition 64/sums -> onat = 32*o_norm),
          transposed via PE identity-matmul, drained as fp8 hi/lo pair
  y       [T, D]    3-term DR against hi/lo W_o (x32), drained as
                    y = psum * 2^-10 + (b_v @ W_o + b_o)

Schedule: single in-order PE stream, software-pipelined per head:
S(h, tk) tiles feed the ACT exp stream; V tiles (heads 0-1 window) and
QK c-tiles (one per head) are interleaved as PE filler; AV(h-2) runs two
heads behind S(h); output projection at the end.
"""
import math
import numpy as np
import concourse.bacc as bacc
import concourse.mybir as mybir
from concourse.tile import TileContext
from concourse.bass import ts

F32 = mybir.dt.float32
F16 = mybir.dt.float16
F8 = mybir.dt.float8e4
AF = mybir.ActivationFunctionType
DR = mybir.MatmulPerfMode.DoubleRow
MULT = mybir.AluOpType.mult
SUB = mybir.AluOpType.subtract

T = 1024       # tokens per core (one batch element)
D = 1024       # d_model
H = 16         # heads
DK = 64        # head dim
NT = T // 128  # 8 token tiles
ND = D // 128  # 8 d tiles
NP = ND // 2   # 4 k-tile pairs for DoubleRow
NC_T = T // 512  # 2 free-dim chunks of tokens
VW = H * (DK + 1)  # 1040, augmented V width
EXP_SCALE = 1.0 / 8192.0      # undo 32*32 on q,k then * 1/sqrt(dk)
EXP_BIAS = -math.log(64.0)    # et = e_true / 64
Y_SCALE = 2.0 ** -10          # undo 32(o) * 32(Wo)


def build_nc(repeat=1):
    nc = bacc.Bacc(None, target_bir_lowering=False, debug=False)

    xhd = nc.dram_tensor("xh", [D, T], F8, kind="ExternalInput")
    xld = nc.dram_tensor("xl", [D, T], F8, kind="ExternalInput")
    wqkh = nc.dram_tensor("wqkh", [D, 2 * D], F8, kind="ExternalInput")
    wqkl = nc.dram_tensor("wqkl", [D, 2 * D], F8, kind="ExternalInput")
    bqkc = nc.dram_tensor("bqkc", [128, 2 * ND], F32, kind="ExternalInput")
    wvhd = nc.dram_tensor("wvh", [D, VW], F8, kind="ExternalInput")
    wvld = nc.dram_tensor("wvl", [D, VW], F8, kind="ExternalInput")
    bvd = nc.dram_tensor("bv", [1, VW], F16, kind="ExternalInput")
    wohd = nc.dram_tensor("woh", [D, D], F8, kind="ExternalInput")
    wold = nc.dram_tensor("wol", [D, D], F8, kind="ExternalInput")
    bord = nc.dram_tensor("bor", [128, D], F32, kind="ExternalInput")
    onesd = nc.dram_tensor("onesd", [1, 128], F16, kind="ExternalInput")
    identd = nc.dram_tensor("identd", [128, 128], F16, kind="ExternalInput")
    ebiasd = nc.dram_tensor("ebias", [128, 1], F32, kind="ExternalInput")
    y = nc.dram_tensor("y", [T, D], F32, kind="ExternalOutput")

    xh_r = xhd.rearrange("(dt p) t -> p dt t", p=128)
    xl_r = xld.rearrange("(dt p) t -> p dt t", p=128)
    wqkh_r = wqkh.rearrange("(dt p) c -> p dt c", p=128)
    wqkl_r = wqkl.rearrange("(dt p) c -> p dt c", p=128)
    wvh_r = wvhd.rearrange("(dt p) c -> p dt c", p=128)
    wvl_r = wvld.rearrange("(dt p) c -> p dt c", p=128)
    woh_r = wohd.rearrange("(dt p) c -> p dt c", p=128)
    wol_r = wold.rearrange("(dt p) c -> p dt c", p=128)

    with TileContext(nc) as tc:
      for _rep in range(repeat):
        with (
            tc.tile_pool(name="res", bufs=1) as res,
            tc.tile_pool(name="wcp", bufs=4) as wcp,
            tc.tile_pool(name="etp", bufs=3) as etp,
            tc.tile_pool(name="onp", bufs=2) as onp,
            tc.tile_pool(name="invp", bufs=2) as invp,
            tc.tile_pool(name="yp", bufs=5) as yp,
            tc.tile_pool(name="ystp", bufs=1) as ystp,
            tc.tile_pool(name="ottp", bufs=2) as ottp,
            tc.tile_pool(name="psW", bufs=2, space="PSUM") as psW,
            tc.tile_pool(name="psAV", bufs=1, space="PSUM") as psAV,
        ):
            # ---- prelude: constants + input DMAs, spread over 4 queues ----
            xth = res.tile([128, ND, T], F8)
            xtl = res.tile([128, ND, T], F8)
            wvth = res.tile([128, ND, VW], F8)
            wvtl = res.tile([128, ND, VW], F8)
            wcsh, wcsl = {}, {}

            def wc_dma(c, q=None):
                qh = q or nc.sync
                wcsh[c] = wcp.tile([128, ND, 128], F8, tag="wqkh",
                                   name=f"wch_{c}")
                wcsl[c] = wcp.tile([128, ND, 128], F8, tag="wqkl",
                                   name=f"wcl_{c}")
                qh.dma_start(wcsh[c][:], wqkh_r[:, :, ts(c, 128)])
                qh.dma_start(wcsl[c][:], wqkl_r[:, :, ts(c, 128)])

            # prelude DMAs on the two HWDGE queues (SP/ACT), interleaved in
            # first-use order; gpsimd uses slow SWDGE (~1us serial setup
            # per DMA) so it only gets non-critical constants
            wcsh[0] = wcp.tile([128, ND, 128], F8, tag="wqkh", name="wch_0")
            wcsl[0] = wcp.tile([128, ND, 128], F8, tag="wqkl", name="wcl_0")
            wcsh[ND] = wcp.tile([128, ND, 128], F8, tag="wqkh",
                                name=f"wch_{ND}")
            wcsl[ND] = wcp.tile([128, ND, 128], F8, tag="wqkl",
                                name=f"wcl_{ND}")
            # hi weights and hi x land first so the opening A-term (hi*hi)
            # matmuls can start as early as possible; lo tensors follow for
            # the B/C correction terms; big x transfers split across the
            # two HWDGE queues to halve queue-serial latency
            nc.sync.dma_start(wcsh[0][:, 0:2, :], wqkh_r[:, 0:2, ts(0, 128)])
            nc.scalar.dma_start(xth[:, 0:2, 0:512], xh_r[:, 0:2, 0:512])
            nc.sync.dma_start(wcsh[0][:, 2:ND, :], wqkh_r[:, 2:ND, ts(0, 128)])
            nc.scalar.dma_start(xth[:, 2:4, 0:512], xh_r[:, 2:4, 0:512])
            nc.sync.dma_start(wcsh[ND][:], wqkh_r[:, :, ts(ND, 128)])
            nc.scalar.dma_start(xth[:, 4:ND, 0:512], xh_r[:, 4:ND, 0:512])
            nc.gpsimd.dma_start(wcsl[0][:], wqkl_r[:, :, ts(0, 128)])
            nc.gpsimd.dma_start(wcsl[ND][:], wqkl_r[:, :, ts(ND, 128)])
            nc.scalar.dma_start(xth[:, 0:4, 512:T], xh_r[:, 0:4, 512:T])
            nc.scalar.dma_start(xth[:, 4:ND, 512:T], xh_r[:, 4:ND, 512:T])
            nc.sync.dma_start(xtl[:, 0:4, 0:512], xl_r[:, 0:4, 0:512])
            nc.sync.dma_start(xtl[:, 4:ND, 0:512], xl_r[:, 4:ND, 0:512])
            nc.scalar.dma_start(xtl[:, 0:4, 512:T], xl_r[:, 0:4, 512:T])
            nc.sync.dma_start(xtl[:, 4:ND, 512:T], xl_r[:, 4:ND, 512:T])
            bqk_t = res.tile([128, 2 * ND], F32)
            bv_t = res.tile([1, VW], F16)
            for d in range(0, ND, 2):
                q = nc.sync if d < 4 else nc.scalar
                q.dma_start(wvth[:, d:d + 2, :], wvh_r[:, d:d + 2, :])
            for d in range(0, ND, 2):
                q = nc.sync if d < 4 else nc.scalar
                q.dma_start(wvtl[:, d:d + 2, :], wvl_r[:, d:d + 2, :])
            wc_dma(1)
            wc_dma(ND + 1)
            nc.gpsimd.dma_start(bqk_t[:], bqkc[:])
            ones_t = res.tile([1, 128], F16)
            nc.gpsimd.dma_start(ones_t[:], onesd[:])
            nc.gpsimd.dma_start(bv_t[:], bvd[:])
            ident_t = res.tile([128, 128], F16)
            nc.gpsimd.dma_start(ident_t[:], identd[:])
            bor_t = res.tile([128, D], F32)
            nc.gpsimd.dma_start(bor_t[:], bord[:])
            ebias_t = res.tile([128, 1], F32)
            nc.gpsimd.dma_start(ebias_t[:], ebiasd[:])

            qk = res.tile([128, 2 * ND, T], F16)
            vg = res.tile([128, NT, VW], F16)
            oth = res.tile([128, ND, T], F8)
            otl = res.tile([128, ND, T], F8)
            wo_th = res.tile([128, ND, D], F8)
            wo_tl = res.tile([128, ND, D], F8)

            # ---- PE work units (quanta ~0.5-1.5us each) ----
            # 3-term hi/lo fp8 DoubleRow product: out += a.T @ b where
            # a ~= ah+al (stationary), b ~= bh+bl (moving), k-pair p.
            def dr3(pp, ah, al, bh, bl, p, asl, bsl, n0, ntot):
                d = 2 * p
                for i, (at, bt) in enumerate(
                        ((ah, bh), (al, bh), (ah, bl))):
                    nc.tensor.matmul(
                        pp, at[:, d:d + 2, asl], bt[:, d:d + 2, bsl],
                        start=(n0 + i == 0), stop=(n0 + i == ntot - 1),
                        perf_mode=DR)

            def qkc_half(c, tq):
                pp = psW.tile([128, 512], F32, tag="half",
                              name=f"pqk_{c}_{tq}")
                for p in range(NP):
                    dr3(pp[:], wcsh[c], wcsl[c], xth, xtl, p,
                        slice(None), ts(tq, 512), 3 * p, 3 * NP)
                nc.vector.tensor_scalar_add(qk[:, c, ts(tq, 512)], pp[:],
                                            bqk_t[:, c:c + 1])

            vchunks = [[(0, 512)], [(512, 512), (1024, VW - 1024)]]

            def vt_half(t, half):
                # softmax rows sum to 1, so the V bias reduces to a constant
                # +bv per output row, folded into the final y bias instead;
                # only the per-head ones columns (for the softmax sums) need
                # the K=1 matmul here, on a 65-strided view
                for off, w in vchunks[half]:
                    pp = psW.tile([128, 512], F32, tag="half",
                                  name=f"pv_{t}_{off}")
                    for p in range(NP):
                        dr3(pp[:, :w], xth, xtl, wvth, wvtl, p,
                            ts(t, 128), slice(off, off + w), 3 * p, 3 * NP)
                    # the ones columns got exactly 0 from the DR loop (their
                    # W columns are zero), so overwrite them as an own group
                    o0 = (64 - off) % 65
                    nc.tensor.matmul(pp[:, o0:w:65], ones_t[:],
                                     bv_t[:, off + o0:off + w:65],
                                     start=True, stop=True,
                                     skip_group_check=True)
                    nc.vector.tensor_copy(vg[:, t, off:off + w], pp[:, :w])

            def s_tile(h, tk, et_h):
                qi, ki = h // 2, ND + h // 2
                b0 = 64 * (h % 2)
                ps = psW.tile([128, 1024], F32, tag="wide",
                              name=f"ps_{h}_{tk}")
                for tq in range(NC_T):
                    nc.tensor.matmul(
                        ps[:, ts(tq, 512)],
                        qk[b0:b0 + DK, ki, ts(tk, 128)],
                        qk[b0:b0 + DK, qi, ts(tq, 512)],
                        start=True, stop=True, tile_position=(b0, 0))
                nc.scalar.activation(et_h[:, tk, :], ps[:], AF.Exp,
                                     scale=EXP_SCALE, bias=ebias_t[:, 0:1])

            onat = {}   # (pair, qt) -> packed O-natural tile (32*o_norm)
            invs = {}   # h -> per-q-token 64/sum tile

            def av_g(h, g, et_h, act_muls=False):
                hp, sub = h // 2, h % 2
                if g == 0:
                    if sub == 0:
                        for qt in range(NT):
                            onat[(hp, qt)] = onp.tile([128, 128], F16,
                                                      tag=f"on{qt}",
                                                      name=f"onat_{hp}_{qt}")
                    invs[h] = invp.tile([128, NT], F32, tag="inv",
                                        name=f"inv_{h}")
                inv = invs[h]
                pool, tag = (psW, "half") if h == 14 else (psAV, f"av{g}")
                pav = pool.tile([128, 512], F32, tag=tag,
                                name=f"pav_{h}_{g}")
                for ql in range(4):
                    qt = g * 4 + ql
                    for tk in range(NT):
                        nc.tensor.matmul(
                            pav[:, ql * 65:(ql + 1) * 65],
                            et_h[:, tk, ts(qt, 128)],
                            vg[:, tk, h * 65:(h + 1) * 65],
                            start=(tk == 0), stop=(tk == NT - 1))
                nc.vector.reciprocal(inv[:, ts(g, 4)],
                                     pav[:, 64:4 * 65:65])
                for ql in range(4):
                    qt = g * 4 + ql
                    dst = onat[(hp, qt)][:, sub * DK:(sub + 1) * DK]
                    src = pav[:, ql * 65:ql * 65 + DK]
                    if act_muls:
                        # pipeline tail: ACT is idle once the last exp is
                        # done, so normalize there and keep DVE free for
                        # the tp drains (Copy shares the Exp act table)
                        nc.scalar.mul(dst, src, inv[:, qt:qt + 1])
                    else:
                        nc.vector.tensor_scalar_mul(dst, src,
                                                    inv[:, qt:qt + 1])

            ysts = {}  # (t, oc) -> staged fp16 partial y (k-pairs 0..1)

            def oproj_pairs(py, t, oc, w, p0, p1, n0, ntot):
                n = n0
                for p in range(p0, p1):
                    dr3(py, oth, otl, wo_th, wo_tl, p,
                        ts(t, 128), slice(oc * 512, oc * 512 + w), n, ntot)
                    n += 3

            deep_yst = set()

            def oproj_partial(t, oc, np1=2):
                ph = psW.tile([128, 512], F32, tag="half",
                              name=f"pyp_{t}_{oc}")
                oproj_pairs(ph[:], t, oc, 512, 0, np1, 0, 3 * np1)
                yst = ystp.tile([128, 512], F16, tag=f"yst{t}_{oc}",
                                name=f"yst_{t}_{oc}")
                nc.vector.tensor_copy(yst[:], ph[:])
                ysts[(t, oc)] = yst
                if np1 == 3:
                    deep_yst.add((t, oc))

            def tp_q(p, g):
                # transpose O-natural pair tiles back to feature-major, then
                # drain the 512-wide result as an fp8 hi/lo pair for the DR
                # output projection.  Early pairs ride the idle DMA engines'
                # xbar transpose (16-bit, 112ns/tile); the last two pairs
                # stay on PE identity-matmuls (shorter latency chain at the
                # pipeline tail): 4 matmuls into ONE psum bank as disjoint
                # 128-col groups, drained once.
                osl = ts(g, 512)
                if p < 6:
                    ott = ottp.tile([128, 512], F16, tag="ott",
                                    name=f"ott_{p}_{g}")
                    for i in range(4):
                        qt = g * 4 + i
                        nc.sync.dma_start_transpose(ott[:, ts(i, 128)],
                                                    onat[(p, qt)][:])
                    src = ott[:]
                else:
                    hold = psAV.tile([128, 512], F32, tag=f"av{g}",
                                     name=f"ptp_{p}_{g}")
                    for i in range(4):
                        qt = g * 4 + i
                        nc.tensor.matmul(hold[:, ts(i, 128)],
                                         onat[(p, qt)][:],
                                         ident_t[:], start=True, stop=True,
                                         skip_group_check=(i > 0))
                    src = hold[:]
                if p >= 7:
                    # pipeline tail: ACT is idle once its exps are done, so
                    # the fp8-hi copy runs there; only the residual subtract
                    # needs DVE
                    nc.scalar.copy(oth[:, p, osl], src)
                else:
                    nc.vector.tensor_copy(oth[:, p, osl], src)
                nc.vector.scalar_tensor_tensor(
                    otl[:, p, osl], src, 1.0, oth[:, p, osl], MULT, SUB)

            # ---- main software-pipelined stream ----
            # per-head slot plans: slots[tk] = list of filler callables
            # emitted right after S(h, tk); emission order == PE order.
            # AV lags two heads behind S (exp of head h-2 is complete);
            # V must fully precede the first AV read of vg.
            def qkc_open():
                # the four opening Q/K psum tiles (c0/c8 x both chunks)
                # accumulate together, emitted in DMA-arrival order: all
                # A-terms (hi*hi) first, then B (w_lo), then C (x_lo), so
                # every arriving transfer immediately feeds matmuls
                pa = psW.tile([128, 512], F32, tag="half", name="pqk_0_0")
                pb = psW.tile([128, 512], F32, tag="half", name=f"pqk_{ND}_0")
                pw = psW.tile([128, 1024], F32, tag="wide", name="pqk_t1")
                t0 = [(pa, 0, 0), (pb, ND, 0)]
                t1 = [(pw[:, 0:512], 0, 1), (pw[:, 512:1024], ND, 1)]
                for group, xt_, w_, st in (
                        (t0, xth, wcsh, True), (t0, xth, wcsl, False),
                        (t1, xth, wcsh, True), (t1, xth, wcsl, False),
                        (t0, xtl, wcsh, False), (t1, xtl, wcsh, False)):
                    for pp, c, tq in group:
                        for p in range(NP):
                            nc.tensor.matmul(
                                pp, w_[c][:, 2 * p:2 * p + 2, :],
                                xt_[:, 2 * p:2 * p + 2, ts(tq, 512)],
                                start=(st and p == 0),
                                stop=(xt_ is xtl and p == NP - 1),
                                perf_mode=DR)
                        if xt_ is xtl:
                            nc.vector.tensor_scalar_add(
                                qk[:, c, ts(tq, 512)], pp, bqk_t[:, c:c + 1])

            qkc_open()
            et_tiles = {}

            def avq(h, g):
                return lambda: av_g(h, g, et_tiles[h])

            def qkq(c, tq):
                return lambda: qkc_half(c, tq)

            def vtq(t, half):
                return lambda: vt_half(t, half)

            def tpq(p, g):
                return lambda: tp_q(p, g)

            def wcq(c):
                return lambda: wc_dma(c)

            def woq(d):
                def f():
                    nc.sync.dma_start(wo_th[:, d:d + 2, :],
                                      woh_r[:, d:d + 2, :])
                    nc.sync.dma_start(wo_tl[:, d:d + 2, :],
                                      wol_r[:, d:d + 2, :])
                return f

            # output-tile order: partials staged h10-h15, finishers at the
            # tail (pair-2 + ident first, pair-3 after tp(7) lands)
            _order = [(t, oc) for t in range(NT) for oc in range(NC_T)]

            def ppq(i, np1=2):
                t, oc = _order[i]
                return lambda: oproj_partial(t, oc, np1)

            opened = {}

            def fin_open(i, py):
                t, oc = _order[i]
                if (t, oc) in deep_yst:
                    nc.tensor.matmul(py, ident_t[:], ysts[(t, oc)][:],
                                     start=True, stop=False)
                else:
                    oproj_pairs(py, t, oc, 512, 2, 3, 0, 99)
                    nc.tensor.matmul(py, ident_t[:], ysts[(t, oc)][:],
                                     start=False, stop=False)
                opened[i] = py

            def foq(i):
                # finisher pair-2 block opened inside h15 on a freed half
                # bank: fills the S(15) exp-latency slots and shortens the
                # post-loop tail
                def f():
                    ph = psW.tile([128, 512], F32, tag="half",
                                  name=f"pfo_{i}")
                    fin_open(i, ph[:])
                return f

            for h in range(H):
                et_tiles[h] = etp.tile([128, NT, T], F16, tag="et",
                                       name=f"et_{h}")
                slots = [[] for _ in range(NT)]
                if h == 0:
                    slots[0] = [qkq(1, 0)]
                    slots[1] = [qkq(1, 1)]
                    slots[2] = [vtq(0, 0)]
                    slots[3] = [vtq(0, 1)]
                    slots[4] = [vtq(1, 0)]
                    slots[5] = [vtq(1, 1)]
                    slots[6] = [vtq(2, 0)]
                    slots[7] = [vtq(2, 1)]
                elif h == 1:
                    slots[0] = [vtq(3, 0)]
                    slots[1] = [vtq(3, 1)]
                    slots[2] = [wcq(2), qkq(ND + 1, 0)]
                    slots[3] = [qkq(ND + 1, 1)]
                    slots[4] = [vtq(4, 0)]
                    slots[5] = [vtq(4, 1)]
                    slots[6] = [vtq(5, 0)]
                    slots[7] = [vtq(5, 1)]
                elif h == 2:
                    slots[0] = [vtq(6, 0)]
                    slots[1] = [vtq(6, 1)]
                    slots[2] = [wcq(ND + 2), vtq(7, 0)]
                    slots[3] = [vtq(7, 1)]
                    slots[4] = [qkq(2, 0)]
                    slots[5] = [qkq(2, 1)]
                    slots[6] = [avq(0, 0)]
                    slots[7] = [avq(0, 1)]
                elif h == 3:
                    slots[0] = [avq(1, 0)]
                    slots[1] = [qkq(ND + 2, 0)]
                    slots[2] = [avq(1, 1)]
                    slots[3] = [qkq(ND + 2, 1)]
                    slots[4] = [tpq(0, 0)]
                    slots[5] = [tpq(0, 1)]
                    slots[6] = [wcq(3)]
                    slots[7] = [wcq(ND + 3)]
                elif h == 4:
                    slots[0] = [avq(2, 0)]
                    slots[1] = [qkq(3, 0)]
                    slots[2] = [avq(2, 1)]
                    slots[3] = [qkq(3, 1)]
                    slots[4] = [qkq(ND + 3, 0)]
                    slots[5] = [wcq(4)]
                    slots[6] = [wcq(ND + 4)]
                elif h == 5:
                    slots[0] = [avq(3, 0)]
                    slots[1] = [qkq(ND + 3, 1)]
                    slots[2] = [avq(3, 1)]
                    slots[3] = [qkq(4, 0)]
                    slots[4] = [tpq(1, 0)]
                    slots[5] = [qkq(4, 1)]
                    slots[6] = [tpq(1, 1)]
                    slots[7] = [wcq(5)]
                elif h == 6:
                    slots[0] = [avq(4, 0)]
                    slots[1] = [qkq(ND + 4, 0)]
                    slots[2] = [avq(4, 1)]
                    slots[3] = [qkq(ND + 4, 1)]
                    slots[4] = [qkq(5, 0)]
                    slots[5] = [wcq(ND + 5)]
                    slots[6] = [woq(0)]
                    slots[7] = []
                elif h == 7:
                    slots[0] = [avq(5, 0)]
                    slots[1] = [qkq(5, 1)]
                    slots[2] = [avq(5, 1)]
                    slots[3] = [qkq(ND + 5, 0)]
                    slots[4] = [tpq(2, 0)]
                    slots[5] = [qkq(ND + 5, 1)]
                    slots[6] = [tpq(2, 1)]
                    slots[7] = [wcq(6), wcq(ND + 6)]
                elif h == 8:
                    slots[0] = [avq(6, 0)]
                    slots[1] = [qkq(6, 0)]
                    slots[2] = [avq(6, 1)]
                    slots[3] = [qkq(6, 1)]
                    slots[4] = [qkq(ND + 6, 0)]
                    slots[5] = [wcq(7)]
                    slots[6] = [woq(2)]
                elif h == 9:
                    slots[0] = [avq(7, 0)]
                    slots[1] = [qkq(ND + 6, 1)]
                    slots[2] = [avq(7, 1)]
                    slots[3] = [qkq(7, 0)]
                    slots[4] = [tpq(3, 0)]
                    slots[5] = [qkq(7, 1)]
                    slots[6] = [tpq(3, 1)]
                    slots[7] = [wcq(ND + 7)]
                elif h == 10:
                    slots[0] = [avq(8, 0)]
                    slots[1] = [qkq(ND + 7, 0)]
                    slots[2] = [avq(8, 1)]
                    slots[3] = [qkq(ND + 7, 1)]
                    slots[4] = [ppq(0)]
                    slots[5] = [ppq(1)]
                    slots[7] = [woq(4)]
                elif h == 11:
                    slots[0] = [avq(9, 0)]
                    slots[1] = [ppq(2)]
                    slots[2] = [avq(9, 1)]
                    slots[3] = [ppq(3)]
                    slots[4] = [tpq(4, 0)]
                    slots[5] = [ppq(4)]
                    slots[6] = [tpq(4, 1)]
                    slots[7] = [woq(6)]
                elif h == 12:
                    slots[0] = [avq(10, 0)]
                    slots[1] = [ppq(5)]
                    slots[2] = [avq(10, 1)]
                    slots[3] = [ppq(6)]
                    slots[4] = [ppq(7)]
                    slots[5] = [ppq(8)]
                elif h == 13:
                    slots[0] = [avq(11, 0)]
                    slots[1] = [ppq(9)]
                    slots[2] = [avq(11, 1)]
                    slots[3] = [ppq(10)]
                    slots[4] = [tpq(5, 0)]
                    slots[5] = [ppq(11)]
                    slots[6] = [tpq(5, 1)]
                elif h == 14:
                    slots[0] = [avq(12, 0)]
                    slots[1] = [ppq(12, 3)]
                    slots[2] = [avq(12, 1)]
                    slots[3] = [ppq(13, 3)]
                    slots[5] = [ppq(14, 3)]
                    slots[7] = [ppq(15, 3)]
                else:  # h == 15
                    slots[0] = [avq(13, 0)]
                    slots[1] = [avq(14, 0)]
                    slots[2] = [avq(13, 1)]
                    slots[3] = [avq(14, 1)]
                    slots[4] = [tpq(6, 0)]
                    slots[6] = [tpq(6, 1)]
                for tk in range(NT):
                    s_tile(h, tk, et_tiles[h])
                    for fn in slots[tk]:
                        fn()

            # ---- pipeline tail + output projection finishers ----
            # av(15) normalizes on the now-idle ACT engine; the first four
            # finishers' pair-2 blocks and ident re-injects run while the
            # av(15) -> tp(7) chain completes, pair-3 closes after
            _dq = [0]

            def fin_drain(t, oc, py):
                yt = yp.tile([128, 512], F32, tag="yt",
                             name=f"yt_{t}_{oc}")
                # scale + bias during the drain (bias includes the folded
                # V-bias term b_v @ W_o); the first half of the finisher
                # stream drains on idle GPSIMD so DVE stays fresh for the
                # span-critical last drains
                nc.vector.scalar_tensor_tensor(
                    yt[:], py, Y_SCALE, bor_t[:, ts(oc, 512)],
                    MULT, mybir.AluOpType.add)
                q = nc.sync if _dq[0] % 2 == 0 else nc.scalar
                _dq[0] += 1
                q.dma_start(y[ts(t, 128), ts(oc, 512)], yt[:])

            def fin_close(i):
                t, oc = _order[i]
                py = opened.pop(i)
                oproj_pairs(py, t, oc, 512, 3, NP, 1, 4)
                fin_drain(t, oc, py)

            def fin_full(i, py):
                t, oc = _order[i]
                if (t, oc) in deep_yst:
                    nc.tensor.matmul(py, ident_t[:], ysts[(t, oc)][:],
                                     start=True, stop=False)
                else:
                    oproj_pairs(py, t, oc, 512, 2, 3, 0, 99)
                    nc.tensor.matmul(py, ident_t[:], ysts[(t, oc)][:],
                                     start=False, stop=False)
                oproj_pairs(py, t, oc, 512, 3, NP, 1, 4)
                fin_drain(t, oc, py)

            def wide_halves(name):
                pw = psW.tile([128, 1024], F32, tag="wide", name=name)
                return pw[:, 0:512], pw[:, 512:1024]

            av_g(H - 1, 0, et_tiles[H - 1], act_muls=True)
            wa0, wa1 = wide_halves("pfin_a")
            fin_open(0, wa0)
            fin_open(1, wa1)
            av_g(H - 1, 1, et_tiles[H - 1], act_muls=True)
            wb0, wb1 = wide_halves("pfin_b")
            fin_open(2, wb0)
            tp_q(H // 2 - 1, 0)
            fin_open(3, wb1)
            tp_q(H // 2 - 1, 1)
            ph0 = psW.tile([128, 512], F32, tag="half", name="pfo_0")
            fin_open(4, ph0[:])
            ph1 = psW.tile([128, 512], F32, tag="half", name="pfo_1")
            fin_open(5, ph1[:])
            for i in range(6):
                fin_close(i)

            def last_tile_split(t, oc):
                # final tile: column halves on INDEPENDENT psum tags
                # (dead wide bank + half bank) so each half's drain+DMA
                # chain pipelines with the other half's matmuls
                yt = yp.tile([128, 512], F32, tag="yt", name="yt_last")
                for hh in range(2):
                    sl = slice(hh * 256, (hh + 1) * 256)
                    co = oc * 512 + hh * 256
                    if hh == 0:
                        ph = psW.tile([128, 1024], F32, tag="wide",
                                      name="pyl_0")[:, 0:256]
                    else:
                        ph = psW.tile([128, 512], F32, tag="half",
                                      name="pyl_1")[:, 0:256]
                    p0 = 3 if (t, oc) in deep_yst else 2
                    n = 0
                    for p in range(p0, NP):
                        d = 2 * p
                        for (ot_, wo_) in ((oth, wo_th), (otl, wo_th),
                                           (oth, wo_tl)):
                            nc.tensor.matmul(
                                ph, ot_[:, d:d + 2, ts(t, 128)],
                                wo_[:, d:d + 2, co:co + 256],
                                start=(n == 0), stop=False, perf_mode=DR)
                            n += 1
                    nc.tensor.matmul(ph, ident_t[:], ysts[(t, oc)][:, sl],
                                     start=False, stop=True)
                    nc.vector.scalar_tensor_tensor(
                        yt[:, sl], ph, Y_SCALE, bor_t[:, co:co + 256],
                        MULT, mybir.AluOpType.add)
                    q = nc.scalar if hh == 0 else nc.sync
                    q.dma_start(y[ts(t, 128), co:co + 256], yt[:, sl])

            for i_ in range(6, len(_order)):
                    if i_ == len(_order) - 1:
                        last_tile_split(*_order[i_])
                        continue
                    # rotate over the dead S-tile banks (wide), the half
                    # banks and the freed AV banks so yt drains never gate
                    # the matmuls
                    k = i_ % 3
                    if k == 0:
                        py = psW.tile([128, 1024], F32, tag="wide",
                                      name=f"pfin_{i_}")[:, 0:512]
                    elif k == 1:
                        py = psW.tile([128, 512], F32, tag="half",
                                      name=f"pfin_{i_}")
                    else:
                        py = psAV.tile([128, 512], F32,
                                       tag=f"av{(i_ // 3) % 2}",
                                       name=f"pfin_{i_}")
                    fin_full(i_, py[:])

    nc.finalize()
    return nc


def prep_in_maps(x, W_qkv, b_qkv, W_o, b_o):
    """Host-side sharding: batch-parallel, one batch element per core.
    Splits x and all weights into fp8e4 hi/lo pairs (weights pre-scaled
    x32 to dodge e4m3 subnormals); computes the folded output bias."""
    F8N = mybir.dt.np(F8)

    def hilo8(a):
        a = np.ascontiguousarray(a, dtype=np.float32)
        hi = a.astype(F8N)
        lo = (a - hi.astype(np.float32)).astype(F8N)
        return hi, lo

    B = x.shape[0]
    W_qk = np.asarray(W_qkv[:, :2 * D], np.float32) * 32.0
    wqk_h, wqk_l = hilo8(W_qk)
    b_qkc = np.ascontiguousarray(
        (np.asarray(b_qkv[:2 * D], np.float32) * 32.0).reshape(2 * ND, 128).T)
    W_vo = np.asarray(W_qkv[:, 2 * D:], np.float32)   # [D, D] V weights
    b_vo = np.asarray(b_qkv[2 * D:], np.float32)
    wv_aug = np.zeros((D, VW), np.float32)
    bv_aug = np.zeros((1, VW), np.float16)
    for h in range(H):
        wv_aug[:, h * (DK + 1):h * (DK + 1) + DK] = \
            W_vo[:, h * DK:(h + 1) * DK] * 32.0
        bv_aug[0, h * (DK + 1) + DK] = 1.0
    wv_h, wv_l = hilo8(wv_aug)
    wo_h, wo_l = hilo8(np.asarray(W_o, np.float32) * 32.0)
    ones = np.ones((1, 128), np.float16)
    ident = np.eye(128, dtype=np.float16)
    ebias = np.full((128, 1), EXP_BIAS, np.float32)
    # folded output bias: y = (o_norm + b_v) @ W_o + b_o
    b_eff = (b_vo @ np.asarray(W_o, np.float32)
             + np.asarray(b_o, np.float32)).astype(np.float32)
    b_or = np.ascontiguousarray(
        np.broadcast_to(b_eff.reshape(1, -1), (128, D)))
    in_maps = []
    for b in range(B):
        xT = np.ascontiguousarray(np.asarray(x[b], np.float32).T)
        xh, xl = hilo8(xT)
        in_maps.append({
            "xh": xh, "xl": xl,
            "wqkh": wqk_h, "wqkl": wqk_l, "bqkc": b_qkc,
            "wvh": wv_h, "wvl": wv_l, "bv": bv_aug,
            "woh": wo_h, "wol": wo_l, "bor": b_or,
            "onesd": ones, "identd": ident, "ebias": ebias,
        })
    return in_maps


# ---------------------------------------------------------------------------
# Self-contained SPMD runner (axon PJRT path) and the graded entry point.
# ---------------------------------------------------------------------------
import jax as _jax


_CACHE = {}


def _make_runner(nc, n_cores=8):
    from jax.sharding import Mesh, PartitionSpec
    from jax.experimental.shard_map import shard_map
    from concourse import bass2jax

    bass2jax.install_neuronx_cc_hook()
    partition_name = nc.partition_id_tensor.name if nc.partition_id_tensor else None
    in_names, out_names, out_avals, zero_outs = [], [], [], []
    for alloc in nc.m.functions[0].allocations:
        if not isinstance(alloc, mybir.MemoryLocationSet):
            continue
        name = alloc.memorylocations[0].name
        if alloc.kind == "ExternalInput":
            if name != partition_name:
                in_names.append(name)
        elif alloc.kind == "ExternalOutput":
            shape = tuple(alloc.tensor_shape)
            dtype = mybir.dt.np(alloc.dtype)
            out_names.append(name)
            out_avals.append(_jax.core.ShapedArray(shape, dtype))
            zero_outs.append(np.zeros(shape, dtype))
    n_params = len(in_names)
    all_in_names = list(in_names) + list(out_names)
    if partition_name is not None:
        all_in_names.append(partition_name)

    def _body(*args):
        operands = list(args)
        if partition_name is not None:
            operands.append(bass2jax.partition_id_tensor())
        return tuple(bass2jax._bass_exec_p.bind(
            *operands,
            out_avals=tuple(out_avals),
            in_names=tuple(all_in_names),
            out_names=tuple(out_names),
            lowering_input_output_aliases=(),
            sim_require_finite=True,
            sim_require_nnan=True,
            nc=nc,
        ))

    devices = _jax.devices()[:n_cores]
    mesh = Mesh(np.asarray(devices), ("core",))
    nin = n_params + len(out_names)
    sharded = _jax.jit(
        shard_map(_body, mesh=mesh,
                  in_specs=(PartitionSpec("core"),) * nin,
                  out_specs=(PartitionSpec("core"),) * len(out_names),
                  check_rep=False),
        keep_unused=True,
    )

    def run(in_maps):
        concat_in = [
            np.concatenate([np.asarray(m[name]) for m in in_maps], axis=0)
            for name in in_names
        ]
        concat_zeros = [
            np.zeros((n_cores * z.shape[0], *z.shape[1:]), z.dtype)
            for z in zero_outs
        ]
        out_arrs = [np.asarray(o) for o in sharded(*concat_in, *concat_zeros)]
        return [
            {name: out_arrs[i].reshape(n_cores, *out_avals[i].shape)[c]
             for i, name in enumerate(out_names)}
            for c in range(n_cores)
        ]

    return run


def kernel(x, W_qkv, b_qkv, W_o, b_o):
    """Full-input entry point: shards batch across the 8 NeuronCores,
    runs the Bass MHA kernel SPMD, gathers the full output."""
    x = np.ascontiguousarray(np.asarray(x, np.float32))
    W_qkv = np.asarray(W_qkv, np.float32)
    b_qkv = np.asarray(b_qkv, np.float32)
    W_o = np.asarray(W_o, np.float32)
    b_o = np.asarray(b_o, np.float32)
    B = x.shape[0]
    assert x.shape == (8, T, D), f"unexpected x shape {x.shape}"

    if "run" not in _CACHE:
        nc = build_nc()
        _CACHE["run"] = _make_runner(nc, n_cores=8)
    run = _CACHE["run"]

    in_maps = prep_in_maps(x, W_qkv, b_qkv, W_o, b_o)
    res = run(in_maps)
    out = np.stack([res[b]["y"] for b in range(B)]).astype(np.float32)
    return out
